# revision 6
# baseline (speedup 1.0000x reference)
"""BoxDecoder (anchor decode + score-threshold filtering) Trainium2 kernel.

Data-parallel over 8 NeuronCores: core k handles batches [2k, 2k+1] of the
leading B=16 dim; anchors are replicated. Each core assembles the dense
[A, C*6] interleaved box tensor in SBUF (coords broadcast over the 8
classes, score column, label column) so every HBM transfer is fully
contiguous, plus the uint8 keep-mask. batch_index is a constant arange,
emitted on host.
"""

import numpy as np

B_TOTAL = 16
A = 100000
C = 8
N_CORES = 8
B_PER = B_TOTAL // N_CORES  # 2

_P = 128
_QMAX = 128  # anchors per partition per tile


def _tile_list(a_pad):
    ts = []
    a0 = 0
    while a0 < a_pad:
        rem = (a_pad - a0) // _P
        q = min(_QMAX, rem)
        ts.append((a0, q))
        a0 += _P * q
    return ts


def _pad_rows(a):
    # round A up so every tile is [128, q] with q <= _QMAX
    full = (a // (_P * _QMAX)) * (_P * _QMAX)
    rem = a - full
    rem_pad = -(-rem // _P) * _P
    return full + rem_pad


A_PAD = _pad_rows(A)  # 100096


def build_program(varx, vary, thresh, a_pad=A_PAD, b_per=B_PER):
    import concourse.mybir as mybir
    from concourse.bacc import Bacc
    from concourse.bass import broadcast_tensor_aps
    from concourse.tile import TileContext
    from contextlib import ExitStack

    f32 = mybir.dt.float32
    u8 = mybir.dt.uint8
    Alu = mybir.AluOpType
    Act = mybir.ActivationFunctionType

    nc = Bacc()
    loc = nc.declare_dram_parameter("loc", [b_per, a_pad, 4], f32, isOutput=False)
    cls_ = nc.declare_dram_parameter("cls", [b_per, a_pad, 8], f32, isOutput=False)
    anch = nc.declare_dram_parameter("anch", [a_pad, 4], f32, isOutput=False)
    boxout = nc.declare_dram_parameter("boxout", [b_per, a_pad, 48], f32, isOutput=True)
    maskout = nc.declare_dram_parameter("maskout", [b_per, a_pad, 8], u8, isOutput=True)

    tiles = _tile_list(a_pad)

    with TileContext(nc) as tc, ExitStack() as ctx:
        anch_pool = ctx.enter_context(tc.tile_pool(name="anch", bufs=2))
        in_pool = ctx.enter_context(tc.tile_pool(name="inp", bufs=3))
        scr = ctx.enter_context(tc.tile_pool(name="scr", bufs=3))
        out_pool = ctx.enter_context(tc.tile_pool(name="outp", bufs=3))

        for a0, q in tiles:
            n = _P * q
            anch_s = anch_pool.tile([_P, 4 * q], f32, tag="anch")
            anch4 = anch_s[:].rearrange("p (q f) -> p q f", f=4)
            nc.sync.dma_start(
                anch4, anch[a0 : a0 + n, :].rearrange("(p q) f -> p q f", p=_P)
            )
            for b in range(b_per):
                loc_s = in_pool.tile([_P, 4 * q], f32, tag="loc")
                loc4 = loc_s[:].rearrange("p (q f) -> p q f", f=4)
                nc.sync.dma_start(
                    loc4, loc[b, a0 : a0 + n, :].rearrange("(p q) f -> p q f", p=_P)
                )
                cls_s = in_pool.tile([_P, 8 * q], f32, tag="cls")
                cls3 = cls_s[:].rearrange("p (q c) -> p q c", c=8)
                nc.sync.dma_start(
                    cls3, cls_[b, a0 : a0 + n, :].rearrange("(p q) c -> p q c", p=_P)
                )

                txy = scr.tile([_P, 2 * q], f32, tag="txy")
                txy2 = txy[:].rearrange("p (q f) -> p q f", f=2)
                xy = scr.tile([_P, 2 * q], f32, tag="xy")
                xy2 = xy[:].rearrange("p (q f) -> p q f", f=2)
                ex = scr.tile([_P, 2 * q], f32, tag="ex")
                ex2 = ex[:].rearrange("p (q f) -> p q f", f=2)
                whh = scr.tile([_P, 2 * q], f32, tag="whh")
                whh2 = whh[:].rearrange("p (q f) -> p q f", f=2)
                co = scr.tile([_P, 4 * q], f32, tag="co")
                co4 = co[:].rearrange("p (q f) -> p q f", f=4)

                # xy = (loc_xy * varx) * anch_wh + anch_xy  (reference op order)
                nc.vector.tensor_scalar_mul(txy2, loc4[:, :, 0:2], varx)
                nc.vector.tensor_tensor(txy2, txy2, anch4[:, :, 2:4], Alu.mult)
                nc.vector.tensor_tensor(xy2, txy2, anch4[:, :, 0:2], Alu.add)
                # e = exp(loc_wh * vary)
                nc.scalar.activation(ex2, loc4[:, :, 2:4], Act.Exp, scale=vary)
                # wh/2 = (e * anch_wh) * 0.5
                nc.vector.tensor_tensor(whh2, ex2, anch4[:, :, 2:4], Alu.mult)
                nc.vector.tensor_scalar_mul(whh2, whh2, 0.5)
                nc.vector.tensor_tensor(co4[:, :, 0:2], xy2, whh2, Alu.subtract)
                nc.vector.tensor_tensor(co4[:, :, 2:4], xy2, whh2, Alu.add)

                out_t = out_pool.tile([_P, 48 * q], f32, tag="out")
                out6 = out_t[:].rearrange("p (q c s) -> p q c s", c=8, s=6)
                # coords broadcast over the 8 classes (stride-0 source read)
                dest = out6[:, :, :, 0:4]
                src = co[:].rearrange("p (q o k) -> p q o k", o=1, k=4)
                d_bc, s_bc = broadcast_tensor_aps(dest, src)
                nc.vector.tensor_copy(d_bc, s_bc)
                # scores column
                nc.scalar.copy(out6[:, :, :, 4], cls3)
                # labels column: c + 1
                nc.gpsimd.iota(
                    out6[:, :, :, 5],
                    pattern=[[0, q], [1, 8]],
                    base=1,
                    channel_multiplier=0,
                    allow_small_or_imprecise_dtypes=True,
                )
                # keep mask
                mask_t = out_pool.tile([_P, 8 * q], u8, tag="mask")
                nc.gpsimd.tensor_scalar(
                    mask_t[:], cls_s[:], thresh, None, Alu.is_ge
                )

                nc.sync.dma_start(
                    boxout[b, a0 : a0 + n, :].rearrange("(p q) s -> p q s", p=_P),
                    out_t[:].rearrange("p (q s) -> p q s", s=48),
                )
                nc.sync.dma_start(
                    maskout[b, a0 : a0 + n, :].rearrange("(p q) c -> p q c", p=_P),
                    mask_t[:].rearrange("p (q c) -> p q c", c=8),
                )
    if not nc.is_finalized():
        nc.finalize()
    return nc


_CACHE = {}


def _get_program(varx, vary, thresh):
    key = (varx, vary, thresh)
    if key not in _CACHE:
        _CACHE[key] = build_program(varx, vary, thresh)
    return _CACHE[key]


def kernel(anchors, loc_preds, cls_preds, varx, vary, score_thresh, batch_size):
    from concourse.bass_utils import run_bass_kernel_spmd

    anchors = np.asarray(anchors, dtype=np.float32)
    loc_preds = np.asarray(loc_preds, dtype=np.float32)
    cls_preds = np.asarray(cls_preds, dtype=np.float32)
    varx = float(varx)
    vary = float(vary)
    thresh = float(score_thresh)

    nc = _get_program(varx, vary, thresh)

    loc_p = np.zeros((B_TOTAL, A_PAD, 4), np.float32)
    loc_p[:, :A] = loc_preds
    cls_p = np.zeros((B_TOTAL, A_PAD, 8), np.float32)
    cls_p[:, :A] = cls_preds
    anch_p = np.zeros((A_PAD, 4), np.float32)
    anch_p[:A] = anchors

    in_maps = [
        {
            "loc": loc_p[k * B_PER : (k + 1) * B_PER],
            "cls": cls_p[k * B_PER : (k + 1) * B_PER],
            "anch": anch_p,
        }
        for k in range(N_CORES)
    ]
    import os

    trace = bool(int(os.environ.get("BOXDEC_TRACE", "0")))
    res = run_bass_kernel_spmd(nc, in_maps, list(range(N_CORES)), trace=trace)
    global _LAST_RESULTS
    _LAST_RESULTS = res

    box = np.empty((B_TOTAL, A, 48), np.float32)
    mask = np.empty((B_TOTAL, A, 8), np.uint8)
    for k in range(N_CORES):
        box[k * B_PER : (k + 1) * B_PER] = res.results[k]["boxout"][:, :A, :]
        mask[k * B_PER : (k + 1) * B_PER] = res.results[k]["maskout"][:, :A, :]

    box_tensor = box.reshape(B_TOTAL * A * C, 6)
    mask_flat = mask.reshape(-1).astype(bool)
    batch_index = np.repeat(np.arange(B_TOTAL, dtype=np.int32), A * C)
    return box_tensor, mask_flat, batch_index


# revision 7
# speedup vs baseline: 1.9284x; 1.9284x over previous
"""BoxDecoder (anchor decode + score-threshold filtering) Trainium2 kernel.

Data-parallel over 8 NeuronCores: core k handles batches [2k, 2k+1] of the
leading B=16 dim; anchors are replicated. Each core assembles the dense
[A, C*6] interleaved box tensor in SBUF and streams it out with fully
contiguous DMAs, plus the uint8 keep-mask. batch_index is a constant
arange, emitted on host.

To keep the compute engines at line rate, the host pre-arranges the
inputs into duplicated channel planes so every elementwise op runs on a
fully contiguous [128, 4q] access pattern:
  locxy = [lx, ly, lx, ly]   locwh = [lw, lh, lw, lh]   per anchor
  mv    = varx*[aw, ah, aw, ah]
  bb    = [ax, ay, ax, ay]
  mwh   = [-aw/2, -ah/2, +aw/2, +ah/2]
Then per anchor the decode is
  E  = exp(locwh * vary)              (ACT, scale folded into activation)
  co = (locxy*mv + bb) + E*mwh        (DVE, contiguous)
giving co = [x1, y1, x2, y2]; the wh half-extent path is bitwise
identical to the reference's (exp(l*vary)*awh)*0.5 (exact *0.5 and sign
folding), the xy path differs only by one product reassociation.
"""

import numpy as np

B_TOTAL = 16
A = 100000
C = 8
N_CORES = 8
B_PER = B_TOTAL // N_CORES  # 2

_P = 128
_QMAX = 128  # anchors per partition per tile


def _tile_list(a_pad):
    ts = []
    a0 = 0
    while a0 < a_pad:
        rem = (a_pad - a0) // _P
        q = min(_QMAX, rem)
        ts.append((a0, q))
        a0 += _P * q
    return ts


def _pad_rows(a):
    full = (a // (_P * _QMAX)) * (_P * _QMAX)
    rem = a - full
    rem_pad = -(-rem // _P) * _P
    return full + rem_pad


A_PAD = _pad_rows(A)  # 100096


def build_program(varx, vary, thresh, a_pad=A_PAD, b_per=B_PER):
    import concourse.mybir as mybir
    from concourse.bacc import Bacc
    from concourse.bass import broadcast_tensor_aps
    from concourse.tile import TileContext
    from contextlib import ExitStack

    f32 = mybir.dt.float32
    u8 = mybir.dt.uint8
    Alu = mybir.AluOpType
    Act = mybir.ActivationFunctionType

    nc = Bacc()
    locxy = nc.declare_dram_parameter("locxy", [b_per, a_pad, 4], f32, isOutput=False)
    locwh = nc.declare_dram_parameter("locwh", [b_per, a_pad, 4], f32, isOutput=False)
    cls_ = nc.declare_dram_parameter("cls", [b_per, a_pad, 8], f32, isOutput=False)
    amv = nc.declare_dram_parameter("amv", [a_pad, 4], f32, isOutput=False)
    abb = nc.declare_dram_parameter("abb", [a_pad, 4], f32, isOutput=False)
    amwh = nc.declare_dram_parameter("amwh", [a_pad, 4], f32, isOutput=False)
    boxout = nc.declare_dram_parameter("boxout", [b_per, a_pad, 48], f32, isOutput=True)
    maskout = nc.declare_dram_parameter("maskout", [b_per, a_pad, 8], u8, isOutput=True)

    tiles = _tile_list(a_pad)

    def load(pool, src, a0, n, q, w, tag):
        # [a0:a0+n, w] dram rows -> [128, q*w] sbuf tile, fully contiguous
        t = pool.tile([_P, w * q], f32, tag=tag)
        nc.sync.dma_start(
            t[:].rearrange("p (q f) -> p q f", f=w),
            src[a0 : a0 + n, :].rearrange("(p q) f -> p q f", p=_P),
        )
        return t

    with TileContext(nc) as tc, ExitStack() as ctx:
        anch_pool = ctx.enter_context(tc.tile_pool(name="anch", bufs=2))
        in_pool = ctx.enter_context(tc.tile_pool(name="inp", bufs=3))
        scr = ctx.enter_context(tc.tile_pool(name="scr", bufs=3))
        out_pool = ctx.enter_context(tc.tile_pool(name="outp", bufs=3))

        for a0, q in tiles:
            n = _P * q
            mv_s = load(anch_pool, amv, a0, n, q, 4, "mv")
            bb_s = load(anch_pool, abb, a0, n, q, 4, "bb")
            mwh_s = load(anch_pool, amwh, a0, n, q, 4, "mwh")
            for b in range(b_per):
                lxy_s = load(in_pool, locxy[b], a0, n, q, 4, "lxy")
                lwh_s = load(in_pool, locwh[b], a0, n, q, 4, "lwh")
                cls_s = in_pool.tile([_P, 8 * q], f32, tag="cls")
                cls3 = cls_s[:].rearrange("p (q c) -> p q c", c=8)
                nc.sync.dma_start(
                    cls3, cls_[b, a0 : a0 + n, :].rearrange("(p q) c -> p q c", p=_P)
                )

                ex = scr.tile([_P, 4 * q], f32, tag="ex")
                w_t = scr.tile([_P, 4 * q], f32, tag="w")
                x_t = scr.tile([_P, 4 * q], f32, tag="x")
                co = scr.tile([_P, 4 * q], f32, tag="co")

                # E = exp(locwh * vary)   (contiguous, scale folded into ACT)
                nc.scalar.activation(ex[:], lwh_s[:], Act.Exp, scale=vary)
                # w = E * [-aw/2, -ah/2, +aw/2, +ah/2]
                nc.vector.tensor_tensor(w_t[:], ex[:], mwh_s[:], Alu.mult)
                # x = locxy * varx*[aw,ah,aw,ah] + [ax,ay,ax,ay]
                nc.vector.tensor_tensor(x_t[:], lxy_s[:], mv_s[:], Alu.mult)
                nc.vector.tensor_tensor(x_t[:], x_t[:], bb_s[:], Alu.add)
                # co = [x1, y1, x2, y2]
                nc.vector.tensor_tensor(co[:], x_t[:], w_t[:], Alu.add)

                out_t = out_pool.tile([_P, 48 * q], f32, tag="out")
                out6 = out_t[:].rearrange("p (q c s) -> p q c s", c=8, s=6)
                co4 = co[:].rearrange("p (q o k) -> p q o k", o=1, k=4)
                # coords broadcast over the 8 classes; split DVE / ACT
                d_lo, s_lo = broadcast_tensor_aps(out6[:, :, 0:5, 0:4], co4)
                nc.vector.tensor_copy(d_lo, s_lo)
                d_hi, s_hi = broadcast_tensor_aps(out6[:, :, 5:8, 0:4], co4)
                nc.scalar.copy(d_hi, s_hi)
                # scores column
                nc.scalar.copy(out6[:, :, :, 4], cls3)
                # labels column: c + 1 (GpSimd is otherwise idle)
                nc.gpsimd.iota(
                    out6[:, :, :, 5],
                    pattern=[[0, q], [1, 8]],
                    base=1,
                    channel_multiplier=0,
                    allow_small_or_imprecise_dtypes=True,
                )
                # keep mask (contiguous f32 -> u8)
                mask_t = out_pool.tile([_P, 8 * q], u8, tag="mask")
                nc.vector.tensor_scalar(mask_t[:], cls_s[:], thresh, None, Alu.is_ge)

                nc.sync.dma_start(
                    boxout[b, a0 : a0 + n, :].rearrange("(p q) s -> p q s", p=_P),
                    out_t[:].rearrange("p (q s) -> p q s", s=48),
                )
                nc.sync.dma_start(
                    maskout[b, a0 : a0 + n, :].rearrange("(p q) c -> p q c", p=_P),
                    mask_t[:].rearrange("p (q c) -> p q c", c=8),
                )
    if not nc.is_finalized():
        nc.finalize()
    return nc


def host_prep(anchors, loc_preds, cls_preds, varx, a_pad=A_PAD):
    """Pad to a_pad rows and build the duplicated channel planes."""
    a = anchors.shape[0]
    b = loc_preds.shape[0]
    locxy = np.zeros((b, a_pad, 4), np.float32)
    locwh = np.zeros((b, a_pad, 4), np.float32)
    locxy[:, :a] = loc_preds[:, :, (0, 1, 0, 1)]
    locwh[:, :a] = loc_preds[:, :, (2, 3, 2, 3)]
    cls_p = np.zeros((b, a_pad, 8), np.float32)
    cls_p[:, :a] = cls_preds

    aw = anchors[:, 2]
    ah = anchors[:, 3]
    ax = anchors[:, 0]
    ay = anchors[:, 1]
    amv = np.zeros((a_pad, 4), np.float32)
    amv[:a] = np.float32(varx) * np.stack([aw, ah, aw, ah], axis=1)
    abb = np.zeros((a_pad, 4), np.float32)
    abb[:a] = np.stack([ax, ay, ax, ay], axis=1)
    amwh = np.zeros((a_pad, 4), np.float32)
    half = np.float32(0.5)
    amwh[:a] = np.stack([-aw * half, -ah * half, aw * half, ah * half], axis=1)
    return locxy, locwh, cls_p, amv, abb, amwh


_CACHE = {}


def _get_program(varx, vary, thresh):
    key = (varx, vary, thresh)
    if key not in _CACHE:
        _CACHE[key] = build_program(varx, vary, thresh)
    return _CACHE[key]


_LAST_RESULTS = None


def kernel(anchors, loc_preds, cls_preds, varx, vary, score_thresh, batch_size):
    from concourse.bass_utils import run_bass_kernel_spmd

    anchors = np.asarray(anchors, dtype=np.float32)
    loc_preds = np.asarray(loc_preds, dtype=np.float32)
    cls_preds = np.asarray(cls_preds, dtype=np.float32)
    varx = float(varx)
    vary = float(vary)
    thresh = float(score_thresh)

    nc = _get_program(varx, vary, thresh)
    locxy, locwh, cls_p, amv, abb, amwh = host_prep(
        anchors, loc_preds, cls_preds, varx
    )

    in_maps = [
        {
            "locxy": locxy[k * B_PER : (k + 1) * B_PER],
            "locwh": locwh[k * B_PER : (k + 1) * B_PER],
            "cls": cls_p[k * B_PER : (k + 1) * B_PER],
            "amv": amv,
            "abb": abb,
            "amwh": amwh,
        }
        for k in range(N_CORES)
    ]
    import os

    trace = bool(int(os.environ.get("BOXDEC_TRACE", "0")))
    res = run_bass_kernel_spmd(nc, in_maps, list(range(N_CORES)), trace=trace)
    global _LAST_RESULTS
    _LAST_RESULTS = res

    box = np.empty((B_TOTAL, A, 48), np.float32)
    mask = np.empty((B_TOTAL, A, 8), np.uint8)
    for k in range(N_CORES):
        box[k * B_PER : (k + 1) * B_PER] = res.results[k]["boxout"][:, :A, :]
        mask[k * B_PER : (k + 1) * B_PER] = res.results[k]["maskout"][:, :A, :]

    box_tensor = box.reshape(B_TOTAL * A * C, 6)
    mask_flat = mask.reshape(-1).astype(bool)
    batch_index = np.repeat(np.arange(B_TOTAL, dtype=np.int32), A * C)
    return box_tensor, mask_flat, batch_index


# revision 8
# speedup vs baseline: 1.9361x; 1.0040x over previous
"""BoxDecoder (anchor decode + score-threshold filtering) Trainium2 kernel.

Data-parallel over 8 NeuronCores: core k handles batches [2k, 2k+1] of the
leading B=16 dim; anchors are replicated. Each core assembles the dense
[A, C*6] interleaved box tensor in SBUF and streams it out with fully
contiguous DMAs, plus the uint8 keep-mask. batch_index is a constant
arange, emitted on host.

The kernel is HBM-bandwidth bound (~40MB of box tensor written per
core), so inputs ship compact and all channel duplication happens
on-chip at line rate:
  loc  [a,4] = [lx, ly, lw, lh]        (as given)
  anc1 [a,4] = [varx*aw, varx*ah, ax, ay]
  shalf[128, 4*q] = per-channel const [-c, -c, +c, +c], c = 0.5/varx
Per tile the engines build duplicated planes
  mv4  = [varx*aw, varx*ah, varx*aw, varx*ah]   (DVE copy, step-0 src)
  bb4  = [ax, ay, ax, ay]                        (DVE copy, step-0 src)
  mwh4 = mv4 * shalf = ~[-aw/2, -ah/2, aw/2, ah/2]
and per batch
  lxy4 = [lx, ly, lx, ly]                        (DVE copy, step-0 src)
  E4   = exp(dup(lw, lh) * vary)                 (ACT, dup via step-0 src)
  co   = (lxy4 * mv4 + bb4) + E4 * mwh4  ->  [x1, y1, x2, y2]
All hot elementwise ops run on fully contiguous [128, 4q] patterns.
"""

import numpy as np

B_TOTAL = 16
A = 100000
C = 8
N_CORES = 8
B_PER = B_TOTAL // N_CORES  # 2

_P = 128
_QMAX = 128  # anchors per partition per tile


def _tile_list(a_pad):
    ts = []
    a0 = 0
    while a0 < a_pad:
        rem = (a_pad - a0) // _P
        q = min(_QMAX, rem)
        ts.append((a0, q))
        a0 += _P * q
    return ts


def _pad_rows(a):
    full = (a // (_P * _QMAX)) * (_P * _QMAX)
    rem = a - full
    rem_pad = -(-rem // _P) * _P
    return full + rem_pad


A_PAD = _pad_rows(A)  # 100096


def build_program(varx, vary, thresh, a_pad=A_PAD, b_per=B_PER):
    import concourse.mybir as mybir
    from concourse.bacc import Bacc
    from concourse.bass import broadcast_tensor_aps
    from concourse.tile import TileContext
    from contextlib import ExitStack

    f32 = mybir.dt.float32
    u8 = mybir.dt.uint8
    Alu = mybir.AluOpType
    Act = mybir.ActivationFunctionType

    nc = Bacc()
    loc = nc.declare_dram_parameter("loc", [b_per, a_pad, 4], f32, isOutput=False)
    cls_ = nc.declare_dram_parameter("cls", [b_per, a_pad, 8], f32, isOutput=False)
    anc1 = nc.declare_dram_parameter("anc1", [a_pad, 4], f32, isOutput=False)
    shalf = nc.declare_dram_parameter("shalf", [_P, 4 * _QMAX], f32, isOutput=False)
    boxout = nc.declare_dram_parameter("boxout", [b_per, a_pad, 48], f32, isOutput=True)
    maskout = nc.declare_dram_parameter("maskout", [b_per, a_pad, 8], u8, isOutput=True)

    tiles = _tile_list(a_pad)

    def dup_pairs(dst_pool, src_tile, q, lo, tag, engine):
        """[.., p0, p1, ..] channel pair -> [p0, p1, p0, p1] per anchor."""
        t = dst_pool.tile([_P, 4 * q], f32, tag=tag)
        dest = t[:].rearrange("p (q r k) -> p q r k", r=2, k=2)
        src = src_tile[:].rearrange("p (q f) -> p q f", f=4)[:, :, lo : lo + 2]
        src = src.rearrange("p q (o k) -> p q o k", o=1)
        d_bc, s_bc = broadcast_tensor_aps(dest, src)
        engine(d_bc, s_bc)
        return t

    with TileContext(nc) as tc, ExitStack() as ctx:
        const_pool = ctx.enter_context(tc.tile_pool(name="const", bufs=1))
        anch_pool = ctx.enter_context(tc.tile_pool(name="anch", bufs=2))
        in_pool = ctx.enter_context(tc.tile_pool(name="inp", bufs=4))
        scr = ctx.enter_context(tc.tile_pool(name="scr", bufs=3))
        out_pool = ctx.enter_context(tc.tile_pool(name="outp", bufs=3))

        shalf_s = const_pool.tile([_P, 4 * _QMAX], f32, tag="shalf")
        nc.sync.dma_start(shalf_s[:], shalf[:, :])

        for a0, q in tiles:
            n = _P * q
            a1_s = anch_pool.tile([_P, 4 * q], f32, tag="a1")
            nc.sync.dma_start(
                a1_s[:].rearrange("p (q f) -> p q f", f=4),
                anc1[a0 : a0 + n, :].rearrange("(p q) f -> p q f", p=_P),
            )
            mv4 = dup_pairs(anch_pool, a1_s, q, 0, "mv4", nc.vector.tensor_copy)
            bb4 = dup_pairs(anch_pool, a1_s, q, 2, "bb4", nc.vector.tensor_copy)
            mwh4 = anch_pool.tile([_P, 4 * q], f32, tag="mwh4")
            nc.vector.tensor_tensor(mwh4[:], mv4[:], shalf_s[:, : 4 * q], Alu.mult)

            for b in range(b_per):
                loc_s = in_pool.tile([_P, 4 * q], f32, tag="loc")
                nc.sync.dma_start(
                    loc_s[:].rearrange("p (q f) -> p q f", f=4),
                    loc[b, a0 : a0 + n, :].rearrange("(p q) f -> p q f", p=_P),
                )
                cls_s = in_pool.tile([_P, 8 * q], f32, tag="cls")
                cls3 = cls_s[:].rearrange("p (q c) -> p q c", c=8)
                nc.sync.dma_start(
                    cls3, cls_[b, a0 : a0 + n, :].rearrange("(p q) c -> p q c", p=_P)
                )

                # lxy4 = [lx, ly, lx, ly]
                lxy4 = dup_pairs(scr, loc_s, q, 0, "lxy4", nc.vector.tensor_copy)
                # E4 = exp(dup(lw, lh) * vary), dup folded into the ACT read
                e4 = scr.tile([_P, 4 * q], f32, tag="e4")
                edst = e4[:].rearrange("p (q r k) -> p q r k", r=2, k=2)
                esrc = loc_s[:].rearrange("p (q f) -> p q f", f=4)[:, :, 2:4]
                esrc = esrc.rearrange("p q (o k) -> p q o k", o=1)
                e_d, e_s = broadcast_tensor_aps(edst, esrc)
                nc.scalar.activation(e_d, e_s, Act.Exp, scale=vary)

                w_t = scr.tile([_P, 4 * q], f32, tag="w")
                x_t = scr.tile([_P, 4 * q], f32, tag="x")
                co = scr.tile([_P, 4 * q], f32, tag="co")
                nc.vector.tensor_tensor(w_t[:], e4[:], mwh4[:], Alu.mult)
                nc.vector.tensor_tensor(x_t[:], lxy4[:], mv4[:], Alu.mult)
                nc.vector.tensor_tensor(x_t[:], x_t[:], bb4[:], Alu.add)
                nc.vector.tensor_tensor(co[:], x_t[:], w_t[:], Alu.add)

                out_t = out_pool.tile([_P, 48 * q], f32, tag="out")
                out6 = out_t[:].rearrange("p (q c s) -> p q c s", c=8, s=6)
                co4 = co[:].rearrange("p (q o k) -> p q o k", o=1, k=4)
                # coords broadcast over the 8 classes; split DVE / ACT
                d_lo, s_lo = broadcast_tensor_aps(out6[:, :, 0:5, 0:4], co4)
                nc.vector.tensor_copy(d_lo, s_lo)
                d_hi, s_hi = broadcast_tensor_aps(out6[:, :, 5:8, 0:4], co4)
                nc.scalar.copy(d_hi, s_hi)
                # scores column
                nc.scalar.copy(out6[:, :, :, 4], cls3)
                # labels column: c + 1 (GpSimd is otherwise idle)
                nc.gpsimd.iota(
                    out6[:, :, :, 5],
                    pattern=[[0, q], [1, 8]],
                    base=1,
                    channel_multiplier=0,
                    allow_small_or_imprecise_dtypes=True,
                )
                # keep mask (contiguous f32 -> u8)
                mask_t = out_pool.tile([_P, 8 * q], u8, tag="mask")
                nc.vector.tensor_scalar(mask_t[:], cls_s[:], thresh, None, Alu.is_ge)

                nc.sync.dma_start(
                    boxout[b, a0 : a0 + n, :].rearrange("(p q) s -> p q s", p=_P),
                    out_t[:].rearrange("p (q s) -> p q s", s=48),
                )
                nc.sync.dma_start(
                    maskout[b, a0 : a0 + n, :].rearrange("(p q) c -> p q c", p=_P),
                    mask_t[:].rearrange("p (q c) -> p q c", c=8),
                )
    if not nc.is_finalized():
        nc.finalize()
    return nc


def host_prep(anchors, loc_preds, cls_preds, varx, a_pad=A_PAD):
    """Pad to a_pad rows; build compact anchor coefficients + shalf const."""
    a = anchors.shape[0]
    b = loc_preds.shape[0]
    loc_p = np.zeros((b, a_pad, 4), np.float32)
    loc_p[:, :a] = loc_preds
    cls_p = np.zeros((b, a_pad, 8), np.float32)
    cls_p[:, :a] = cls_preds

    anc1 = np.zeros((a_pad, 4), np.float32)
    anc1[:a, 0] = np.float32(varx) * anchors[:, 2]
    anc1[:a, 1] = np.float32(varx) * anchors[:, 3]
    anc1[:a, 2] = anchors[:, 0]
    anc1[:a, 3] = anchors[:, 1]

    c = np.float32(0.5) / np.float32(varx)
    shalf = np.tile(
        np.array([-c, -c, c, c], np.float32), _QMAX
    )[None, :].repeat(_P, axis=0)
    return loc_p, cls_p, anc1, np.ascontiguousarray(shalf)


_CACHE = {}


def _get_program(varx, vary, thresh):
    key = (varx, vary, thresh)
    if key not in _CACHE:
        _CACHE[key] = build_program(varx, vary, thresh)
    return _CACHE[key]


_LAST_RESULTS = None


def kernel(anchors, loc_preds, cls_preds, varx, vary, score_thresh, batch_size):
    from concourse.bass_utils import run_bass_kernel_spmd

    anchors = np.asarray(anchors, dtype=np.float32)
    loc_preds = np.asarray(loc_preds, dtype=np.float32)
    cls_preds = np.asarray(cls_preds, dtype=np.float32)
    varx = float(varx)
    vary = float(vary)
    thresh = float(score_thresh)

    nc = _get_program(varx, vary, thresh)
    loc_p, cls_p, anc1, shalf = host_prep(anchors, loc_preds, cls_preds, varx)

    in_maps = [
        {
            "loc": loc_p[k * B_PER : (k + 1) * B_PER],
            "cls": cls_p[k * B_PER : (k + 1) * B_PER],
            "anc1": anc1,
            "shalf": shalf,
        }
        for k in range(N_CORES)
    ]
    import os

    trace = bool(int(os.environ.get("BOXDEC_TRACE", "0")))
    res = run_bass_kernel_spmd(nc, in_maps, list(range(N_CORES)), trace=trace)
    global _LAST_RESULTS
    _LAST_RESULTS = res

    box = np.empty((B_TOTAL, A, 48), np.float32)
    mask = np.empty((B_TOTAL, A, 8), np.uint8)
    for k in range(N_CORES):
        box[k * B_PER : (k + 1) * B_PER] = res.results[k]["boxout"][:, :A, :]
        mask[k * B_PER : (k + 1) * B_PER] = res.results[k]["maskout"][:, :A, :]

    box_tensor = box.reshape(B_TOTAL * A * C, 6)
    mask_flat = mask.reshape(-1).astype(bool)
    batch_index = np.repeat(np.arange(B_TOTAL, dtype=np.int32), A * C)
    return box_tensor, mask_flat, batch_index


# revision 9
# speedup vs baseline: 2.1341x; 1.1022x over previous
"""BoxDecoder (anchor decode + score-threshold filtering) Trainium2 kernel.

Data-parallel over 8 NeuronCores: core k handles batches [2k, 2k+1] of the
leading B=16 dim; anchors are replicated. Each core assembles the dense
[A, C*6] interleaved box tensor in SBUF and streams it out with fully
contiguous DMAs, plus the uint8 keep-mask. batch_index is a constant
arange, emitted on host.

The kernel is HBM-bandwidth bound (~40MB of box tensor written per
core), so inputs ship compact and all channel duplication happens
on-chip at line rate:
  loc  [a,4] = [lx, ly, lw, lh]        (as given)
  anc1 [a,4] = [varx*aw, varx*ah, ax, ay]
  shalf[128, 4*q] = per-channel const [-c, -c, +c, +c], c = 0.5/varx
Per tile the engines build duplicated planes
  mv4  = [varx*aw, varx*ah, varx*aw, varx*ah]   (DVE copy, step-0 src)
  bb4  = [ax, ay, ax, ay]                        (DVE copy, step-0 src)
  mwh4 = mv4 * shalf = ~[-aw/2, -ah/2, aw/2, ah/2]
and per batch
  lxy4 = [lx, ly, lx, ly]                        (DVE copy, step-0 src)
  E4   = exp(dup(lw, lh) * vary)                 (ACT, dup via step-0 src)
  co   = (lxy4 * mv4 + bb4) + E4 * mwh4  ->  [x1, y1, x2, y2]
All hot elementwise ops run on fully contiguous [128, 4q] patterns.
"""

import numpy as np

B_TOTAL = 16
A = 100000
C = 8
N_CORES = 8
B_PER = B_TOTAL // N_CORES  # 2

_P = 128
_QMAX = 128  # anchors per partition per tile


def _tile_list(a_pad):
    ts = []
    a0 = 0
    while a0 < a_pad:
        rem = (a_pad - a0) // _P
        q = min(_QMAX, rem)
        ts.append((a0, q))
        a0 += _P * q
    return ts


def _pad_rows(a):
    full = (a // (_P * _QMAX)) * (_P * _QMAX)
    rem = a - full
    rem_pad = -(-rem // _P) * _P
    return full + rem_pad


A_PAD = _pad_rows(A)  # 100096


def build_program(varx, vary, thresh, a_pad=A_PAD, b_per=B_PER):
    import concourse.mybir as mybir
    from concourse.bacc import Bacc
    from concourse.bass import broadcast_tensor_aps
    from concourse.tile import TileContext
    from contextlib import ExitStack

    f32 = mybir.dt.float32
    u8 = mybir.dt.uint8
    Alu = mybir.AluOpType
    Act = mybir.ActivationFunctionType

    nc = Bacc()
    loc = nc.declare_dram_parameter("loc", [b_per, a_pad, 4], f32, isOutput=False)
    cls_ = nc.declare_dram_parameter("cls", [b_per, a_pad, 8], f32, isOutput=False)
    anc1 = nc.declare_dram_parameter("anc1", [a_pad, 4], f32, isOutput=False)
    shalf = nc.declare_dram_parameter("shalf", [_P, 4 * _QMAX], f32, isOutput=False)
    boxout = nc.declare_dram_parameter("boxout", [b_per, a_pad, 48], f32, isOutput=True)
    maskout = nc.declare_dram_parameter("maskout", [b_per, a_pad, 8], u8, isOutput=True)

    tiles = _tile_list(a_pad)

    def dup_pairs(dst_pool, src_tile, q, lo, tag, engine):
        """[.., p0, p1, ..] channel pair -> [p0, p1, p0, p1] per anchor."""
        t = dst_pool.tile([_P, 4 * q], f32, tag=tag)
        dest = t[:].rearrange("p (q r k) -> p q r k", r=2, k=2)
        src = src_tile[:].rearrange("p (q f) -> p q f", f=4)[:, :, lo : lo + 2]
        src = src.rearrange("p q (o k) -> p q o k", o=1)
        d_bc, s_bc = broadcast_tensor_aps(dest, src)
        engine(d_bc, s_bc)
        return t

    with TileContext(nc) as tc, ExitStack() as ctx:
        const_pool = ctx.enter_context(tc.tile_pool(name="const", bufs=1))
        anch_pool = ctx.enter_context(tc.tile_pool(name="anch", bufs=2))
        in_pool = ctx.enter_context(tc.tile_pool(name="inp", bufs=4))
        scr = ctx.enter_context(tc.tile_pool(name="scr", bufs=4))
        out_pool = ctx.enter_context(tc.tile_pool(name="outp", bufs=4))

        shalf_s = const_pool.tile([_P, 4 * _QMAX], f32, tag="shalf")
        nc.scalar.dma_start(shalf_s[:], shalf[:, :])

        for a0, q in tiles:
            n = _P * q
            a1_s = anch_pool.tile([_P, 4 * q], f32, tag="a1")
            nc.scalar.dma_start(
                a1_s[:].rearrange("p (q f) -> p q f", f=4),
                anc1[a0 : a0 + n, :].rearrange("(p q) f -> p q f", p=_P),
            )
            mv4 = dup_pairs(anch_pool, a1_s, q, 0, "mv4", nc.vector.tensor_copy)
            bb4 = dup_pairs(anch_pool, a1_s, q, 2, "bb4", nc.vector.tensor_copy)
            mwh4 = anch_pool.tile([_P, 4 * q], f32, tag="mwh4")
            nc.vector.tensor_tensor(mwh4[:], mv4[:], shalf_s[:, : 4 * q], Alu.mult)

            for b in range(b_per):
                loc_s = in_pool.tile([_P, 4 * q], f32, tag="loc")
                nc.scalar.dma_start(
                    loc_s[:].rearrange("p (q f) -> p q f", f=4),
                    loc[b, a0 : a0 + n, :].rearrange("(p q) f -> p q f", p=_P),
                )
                cls_s = in_pool.tile([_P, 8 * q], f32, tag="cls")
                cls3 = cls_s[:].rearrange("p (q c) -> p q c", c=8)
                nc.scalar.dma_start(
                    cls3, cls_[b, a0 : a0 + n, :].rearrange("(p q) c -> p q c", p=_P)
                )

                # lxy4 = [lx, ly, lx, ly]
                lxy4 = dup_pairs(scr, loc_s, q, 0, "lxy4", nc.vector.tensor_copy)
                # E4 = exp(dup(lw, lh) * vary), dup folded into the ACT read
                e4 = scr.tile([_P, 4 * q], f32, tag="e4")
                edst = e4[:].rearrange("p (q r k) -> p q r k", r=2, k=2)
                esrc = loc_s[:].rearrange("p (q f) -> p q f", f=4)[:, :, 2:4]
                esrc = esrc.rearrange("p q (o k) -> p q o k", o=1)
                e_d, e_s = broadcast_tensor_aps(edst, esrc)
                nc.scalar.activation(e_d, e_s, Act.Exp, scale=vary)

                w_t = scr.tile([_P, 4 * q], f32, tag="w")
                x_t = scr.tile([_P, 4 * q], f32, tag="x")
                co = scr.tile([_P, 4 * q], f32, tag="co")
                nc.vector.tensor_tensor(w_t[:], e4[:], mwh4[:], Alu.mult)
                nc.vector.tensor_tensor(x_t[:], lxy4[:], mv4[:], Alu.mult)
                nc.vector.tensor_tensor(x_t[:], x_t[:], bb4[:], Alu.add)
                nc.vector.tensor_tensor(co[:], x_t[:], w_t[:], Alu.add)

                out_t = out_pool.tile([_P, 48 * q], f32, tag="out")
                out6 = out_t[:].rearrange("p (q c s) -> p q c s", c=8, s=6)
                co4 = co[:].rearrange("p (q o k) -> p q o k", o=1, k=4)
                # coords broadcast over the 8 classes; split DVE / ACT
                d_lo, s_lo = broadcast_tensor_aps(out6[:, :, 0:5, 0:4], co4)
                nc.vector.tensor_copy(d_lo, s_lo)
                d_hi, s_hi = broadcast_tensor_aps(out6[:, :, 5:8, 0:4], co4)
                nc.scalar.copy(d_hi, s_hi)
                # scores column
                nc.scalar.copy(out6[:, :, :, 4], cls3)
                # labels column: c + 1 (GpSimd is otherwise idle)
                nc.gpsimd.iota(
                    out6[:, :, :, 5],
                    pattern=[[0, q], [1, 8]],
                    base=1,
                    channel_multiplier=0,
                    allow_small_or_imprecise_dtypes=True,
                )
                # keep mask (contiguous f32 -> u8)
                mask_t = out_pool.tile([_P, 8 * q], u8, tag="mask")
                nc.vector.tensor_scalar(mask_t[:], cls_s[:], thresh, None, Alu.is_ge)

                nc.sync.dma_start(
                    boxout[b, a0 : a0 + n, :].rearrange("(p q) s -> p q s", p=_P),
                    out_t[:].rearrange("p (q s) -> p q s", s=48),
                )
                nc.sync.dma_start(
                    maskout[b, a0 : a0 + n, :].rearrange("(p q) c -> p q c", p=_P),
                    mask_t[:].rearrange("p (q c) -> p q c", c=8),
                )
    if not nc.is_finalized():
        nc.finalize()
    return nc


def host_prep(anchors, loc_preds, cls_preds, varx, a_pad=A_PAD):
    """Pad to a_pad rows; build compact anchor coefficients + shalf const."""
    a = anchors.shape[0]
    b = loc_preds.shape[0]
    loc_p = np.zeros((b, a_pad, 4), np.float32)
    loc_p[:, :a] = loc_preds
    cls_p = np.zeros((b, a_pad, 8), np.float32)
    cls_p[:, :a] = cls_preds

    anc1 = np.zeros((a_pad, 4), np.float32)
    anc1[:a, 0] = np.float32(varx) * anchors[:, 2]
    anc1[:a, 1] = np.float32(varx) * anchors[:, 3]
    anc1[:a, 2] = anchors[:, 0]
    anc1[:a, 3] = anchors[:, 1]

    c = np.float32(0.5) / np.float32(varx)
    shalf = np.tile(
        np.array([-c, -c, c, c], np.float32), _QMAX
    )[None, :].repeat(_P, axis=0)
    return loc_p, cls_p, anc1, np.ascontiguousarray(shalf)


_CACHE = {}


def _get_program(varx, vary, thresh):
    key = (varx, vary, thresh)
    if key not in _CACHE:
        _CACHE[key] = build_program(varx, vary, thresh)
    return _CACHE[key]


_LAST_RESULTS = None


def kernel(anchors, loc_preds, cls_preds, varx, vary, score_thresh, batch_size):
    from concourse.bass_utils import run_bass_kernel_spmd

    anchors = np.asarray(anchors, dtype=np.float32)
    loc_preds = np.asarray(loc_preds, dtype=np.float32)
    cls_preds = np.asarray(cls_preds, dtype=np.float32)
    varx = float(varx)
    vary = float(vary)
    thresh = float(score_thresh)

    nc = _get_program(varx, vary, thresh)
    loc_p, cls_p, anc1, shalf = host_prep(anchors, loc_preds, cls_preds, varx)

    in_maps = [
        {
            "loc": loc_p[k * B_PER : (k + 1) * B_PER],
            "cls": cls_p[k * B_PER : (k + 1) * B_PER],
            "anc1": anc1,
            "shalf": shalf,
        }
        for k in range(N_CORES)
    ]
    import os

    trace = bool(int(os.environ.get("BOXDEC_TRACE", "0")))
    res = run_bass_kernel_spmd(nc, in_maps, list(range(N_CORES)), trace=trace)
    global _LAST_RESULTS
    _LAST_RESULTS = res

    box = np.empty((B_TOTAL, A, 48), np.float32)
    mask = np.empty((B_TOTAL, A, 8), np.uint8)
    for k in range(N_CORES):
        box[k * B_PER : (k + 1) * B_PER] = res.results[k]["boxout"][:, :A, :]
        mask[k * B_PER : (k + 1) * B_PER] = res.results[k]["maskout"][:, :A, :]

    box_tensor = box.reshape(B_TOTAL * A * C, 6)
    mask_flat = mask.reshape(-1).astype(bool)
    batch_index = np.repeat(np.arange(B_TOTAL, dtype=np.int32), A * C)
    return box_tensor, mask_flat, batch_index
